# revision 28
# baseline (speedup 1.0000x reference)
"""Trainium2 Bass kernel for DilateAttention (3x3 kernel, dilation 2).

q,k,v: [B=4, d=384, H=64, W=64] f32.  heads=12, head_dim=32.
out: [B, H, W, d] f32.  Core = (batch b, row-half); 8 cores.

v3 design (vs v2 baseline):
  - bc (attn broadcast 36 rows -> 128 rows) moved OFF PE+ACT onto
    replicating SBUF->SBUF DMAs: one dma per (half-group, ko) with a
    forged source AP [[9,4],[0,32],[1,1024]] that repeats each head row
    32x across dest partitions.  Kills 108 PE matmuls and 36 slow
    PSUM->SBUF ACT copies (~51us of ACT at 1x f32).
  - ksum fully on PE (identity accumulate, 9 MMs/chunk) -> ACT copy.
  - softmax of group g+1 fully interleaved into the AV phase of group g;
    per chunk: prods(g+1) DVE, logits(g+1) PE, pav(g) DVE, ksum(g) PE.
  - bc DMAs issued on the scalar (ACT) HWDGE ring; loads/stores on sync.
  - output: channel-major bf16 [384, 2048] per core UNNORMALIZED + z on
    host (same contract as v2).
"""

import os
import sys

for _p in ("/opt/trn_rl_repo", "/root/.axon_site/_ro/trn_rl_repo"):
    if _p not in sys.path and os.path.isdir(_p):
        sys.path.insert(0, _p)

import dataclasses
from contextlib import ExitStack

import numpy as np
import ml_dtypes

import concourse.bass as bass
import concourse.bacc as bacc
import concourse.mybir as mybir
import concourse.tile as tile
from concourse import masks
from concourse.bass_utils import run_bass_kernel_spmd

BF16 = ml_dtypes.bfloat16

B, D, H, W = 4, 384, 64, 64
NH, HD = 12, 32
KK, DIL, PAD = 3, 2, 2
K2 = KK * KK
SCALE = HD ** -0.5

NCORES = 8
ROWS = H // 2              # 32 output rows per core
HROWS = ROWS + 2 * PAD     # 36 halo rows of padded k/v
WP = W + 2 * PAD           # 68 padded width
NGRP = 3                   # channel groups of 128 partitions
HPG = 4                    # heads per group
FD = ROWS * W              # 2048 pixels per core
NCH = 4                    # chunks
CHD = FD // NCH            # 512 chunk free dim
HFD = FD // 2              # 1024 half free dim
LG = HPG * K2              # 36 logit rows per group
QR = ROWS // NCH           # 8 rows per quarter

# offsets broadcast on PE+ACT vs replicating SWDGE DMA
PE_KOS = (0, 4, 8)
DMA_KOS = (1, 2, 3, 5, 6, 7)

_CACHE = {}


def _build_sel_constants():
    # logits reduce: for offset ko, lhsT[p=hl*32+c, m=ko*4+hl] = 1 (M=36)
    # (ko-major rows so one bc DMA reads 4 consecutive partitions)
    selqk = np.zeros((128, K2, LG), np.float32)
    for hl in range(HPG):
        for c in range(HD):
            for ko in range(K2):
                selqk[hl * HD + c, ko, ko * HPG + hl] = 1.0
    # Z: zp[hl, l] = sum_ko exp[ko*4+hl, l]
    selz = np.zeros((LG, HPG), np.float32)
    for hl in range(HPG):
        for ko in range(K2):
            selz[ko * HPG + hl, hl] = 1.0
    # bc broadcast for PE-routed offsets: lhsT[p=ko*4+hl, m=hl*32+c] = 1
    selbc = np.zeros((LG, len(PE_KOS), 128), np.float32)
    for i, ko in enumerate(PE_KOS):
        for hl in range(HPG):
            for c in range(HD):
                selbc[ko * HPG + hl, i, hl * HD + c] = 1.0
    return (
        selqk.reshape(128, K2 * LG).astype(BF16),
        selz.astype(BF16),
        selbc.reshape(LG, len(PE_KOS) * 128).astype(BF16),
    )


def _build_nc():
    nc = bacc.Bacc("TRN2", target_bir_lowering=False, debug=False,
                   num_devices=NCORES)
    f32 = mybir.dt.float32
    bf16 = mybir.dt.bfloat16

    q_p = nc.declare_dram_parameter("q", [D, FD], bf16, isOutput=False)
    k_p = nc.declare_dram_parameter("k", [D, HROWS * WP], bf16, isOutput=False)
    v_p = nc.declare_dram_parameter("v", [D, HROWS * WP], bf16, isOutput=False)
    selqk_p = nc.declare_dram_parameter("selqk", [128, K2 * LG], bf16, isOutput=False)
    selz_p = nc.declare_dram_parameter("selz", [LG, HPG], bf16, isOutput=False)
    selbc_p = nc.declare_dram_parameter("selbc", [LG, len(PE_KOS) * 128], bf16,
                                        isOutput=False)
    out_p = nc.declare_dram_parameter("out", [D, FD], bf16, isOutput=True)
    z_p = nc.declare_dram_parameter("z", [NGRP * HPG, FD], f32, isOutput=True)

    with tile.TileContext(nc) as tc, ExitStack() as ctx:
        consts = ctx.enter_context(tc.tile_pool(name="consts", bufs=1))
        inp = ctx.enter_context(tc.tile_pool(name="inp", bufs=2))
        prods = ctx.enter_context(tc.tile_pool(name="prods", bufs=2))
        smax = ctx.enter_context(tc.tile_pool(name="smax", bufs=2))
        bcp = ctx.enter_context(tc.tile_pool(name="bcp", bufs=3))
        accp = ctx.enter_context(tc.tile_pool(name="accp", bufs=2))
        ps_sm = ctx.enter_context(tc.tile_pool(name="ps_sm", bufs=2, space="PSUM"))
        ps_acc = ctx.enter_context(tc.tile_pool(name="ps_acc", bufs=2, space="PSUM"))
        ps_bc = ctx.enter_context(tc.tile_pool(name="ps_bc", bufs=4, space="PSUM"))

        selqk_t = consts.tile([128, K2 * LG], bf16)
        nc.sync.dma_start(selqk_t[:], selqk_p[:])
        selz_t = consts.tile([LG, HPG], bf16)
        nc.sync.dma_start(selz_t[:], selz_p[:])
        selbc_t = consts.tile([LG, len(PE_KOS) * 128], bf16)
        nc.sync.dma_start(selbc_t[:], selbc_p[:])
        ident = consts.tile([128, 128], bf16)
        masks.make_identity(nc, ident[:])

        KSPLIT = 20 * WP

        def load_group(g):
            gp = slice(g * 128, (g + 1) * 128)
            q_t = inp.tile([128, FD], bf16, tag="q", name="q_t")
            k_t = inp.tile([128, HROWS * WP], bf16, tag="k", name="k_t")
            nc.sync.dma_start(q_t[:, :FD // 2], q_p[gp, :FD // 2])
            nc.sync.dma_start(k_t[:, :KSPLIT], k_p[gp, :KSPLIT])
            nc.sync.dma_start(q_t[:, FD // 2:], q_p[gp, FD // 2:])
            nc.sync.dma_start(k_t[:, KSPLIT:], k_p[gp, KSPLIT:])
            v_t = inp.tile([128, HROWS * WP], bf16, tag="v", name="v_t")
            nc.sync.dma_start(v_t[:, :KSPLIT], v_p[gp, :KSPLIT])
            nc.sync.dma_start(v_t[:, KSPLIT:], v_p[gp, KSPLIT:])
            k3 = k_t[:].rearrange("p (r w) -> p r w", r=HROWS)
            v3 = v_t[:].rearrange("p (r w) -> p r w", r=HROWS)
            return q_t, k3, v3

        def dj_triple(t3, di, r0, nr):
            sl = t3[:, DIL * di + r0: DIL * di + r0 + nr, 0:W]
            return dataclasses.replace(
                sl, ap=[sl.ap[0], [DIL, KK]] + list(sl.ap[1:]))

        def rep3(flat, nr):
            sl = flat.rearrange("p (r w) -> p r w", r=nr)
            return dataclasses.replace(
                sl, ap=[sl.ap[0], [0, KK]] + list(sl.ap[1:]))

        def alloc_prods():
            pts = [prods.tile([128, KK * FD], bf16, tag=f"pd{di}",
                              name=f"pd{di}") for di in range(KK)]
            ptiles = []
            for di in range(KK):
                ptiles.extend(pts[di][:, dj * FD:(dj + 1) * FD]
                              for dj in range(KK))
            return pts, ptiles

        def emit_prods_half(q_t, k3, pts, half):
            # products for row-half (pixels half*1024..+1024), all 9 ko
            hr = ROWS // 2
            for di in range(KK):
                pt4 = pts[di][:].rearrange("p (k r w) -> p k r w",
                                           k=KK, r=ROWS)
                nc.vector.tensor_mul(
                    pt4[:, :, half * hr:(half + 1) * hr, :],
                    rep3(q_t[:, half * hr * W:(half + 1) * hr * W], hr),
                    dj_triple(k3, di, half * hr, hr),
                )

        def emit_softmax_chunk(ptiles, exp_t, zsb, bc_t, ch):
            # logits (9 acc MMs) -> exp -> Z (normalization on host)
            # + PE-routed bc broadcast for this chunk
            cs = slice(ch * CHD, (ch + 1) * CHD)
            hc = (ch % 2) * CHD
            lg = ps_sm.tile([LG, CHD], f32, tag="sm", name="lg")
            for ko in range(K2):
                nc.tensor.matmul(
                    lg[:], selqk_t[:, ko * LG:(ko + 1) * LG],
                    ptiles[ko][:, cs],
                    start=(ko == 0), stop=(ko == K2 - 1),
                )
            nc.scalar.activation(
                exp_t[0:LG, cs], lg[:],
                mybir.ActivationFunctionType.Exp, scale=float(SCALE),
            )
            zp = ps_sm.tile([HPG, CHD], f32, tag="sm", name="zp")
            nc.tensor.matmul(zp[:], selz_t[:], exp_t[0:LG, cs],
                             start=True, stop=True)
            nc.scalar.copy(zsb[:, cs], zp[:])
            for i, ko in enumerate(PE_KOS):
                bcps = ps_bc.tile([128, CHD], f32, tag="bcps", name="bcps")
                nc.tensor.matmul(
                    bcps[:], selbc_t[:, i * 128:(i + 1) * 128],
                    exp_t[0:LG, cs], start=True, stop=True)
                nc.scalar.copy(bc_t[:, ko * HFD + hc: ko * HFD + hc + CHD],
                               bcps[:])

        BCP9 = K2 * HFD  # bc tile pitch in elements

        def emit_bc_half(exp_t, bc_t, half):
            # replicate exp rows 32x via SWDGE DMAs for the DMA-routed kos.
            # exp rows are first mirrored to partitions 64..99 (odd AXI
            # ports) and each broadcast is split into two 16-replica DMAs
            # so source reads use twice the SBUF ports.
            c0 = half * HFD
            if half == 1:
                # mirror exp rows to partitions 64.. (odd AXI ports) so
                # each ko's replicated reads split across two ports
                nc.gpsimd.dma_start(exp_t[64:64 + LG, c0:c0 + HFD],
                                    exp_t[0:LG, c0:c0 + HFD])
            p0 = 64 * half
            for ko in DMA_KOS:
                src = exp_t[p0 + ko * HPG: p0 + ko * HPG + HPG, c0:c0 + HFD]
                src = dataclasses.replace(
                    src, ap=[[FD, HPG], [0, HD], [1, HFD]])
                nc.gpsimd.dma_start(
                    bc_t[:, ko * HFD:(ko + 1) * HFD], src)

        def emit_pav(bc_t, v3, ch):
            # pav[di] = bc x v_shift on DVE; returns 9 [128, CHD] slices
            hc = (ch % 2) * CHD
            r0 = ch * QR
            pavs = []
            for di in range(KK):
                bc3 = bc_t[:].rearrange("p (k f) -> p k f", k=K2)
                bcv = bc3[:, di * KK:(di + 1) * KK, hc:hc + CHD]
                bcv4 = bcv.rearrange("p k (r w) -> p k r w", r=QR)
                pav = accp.tile([128, KK * CHD], bf16, tag=f"pav{di}",
                                name=f"pav{di}")
                nc.vector.tensor_mul(
                    pav[:].rearrange("p (k r w) -> p k r w", k=KK, r=QR),
                    bcv4,
                    dj_triple(v3, di, r0, QR),
                )
                pavs.extend(pav[:, dj * CHD:(dj + 1) * CHD]
                            for dj in range(KK))
            return pavs

        def emit_ksum(acc_t, ch, pavs):
            cs = slice(ch * CHD, (ch + 1) * CHD)
            acc_ps = ps_acc.tile([128, CHD], f32, tag="acc", name="acc_ps")
            for ko in range(K2):
                nc.tensor.matmul(acc_ps[:], ident[:], pavs[ko],
                                 start=(ko == 0), stop=(ko == K2 - 1))
            nc.scalar.copy(acc_t[:, cs], acc_ps[:])

        # prime group 0: products + full softmax + both bc halves
        tiles = load_group(0)
        pts, ptiles = alloc_prods()
        # PE warm-up: keep the HAM activity monitor busy through the
        # products phase so the first logits matmuls run at 2.4 GHz
        warm = ps_acc.tile([128, K2 * LG], f32, tag="acc", name="warm")
        for _ in range(64):
            nc.tensor.matmul(warm[:], ident[:], selqk_t[:],
                             start=True, stop=True)
        exp_t = smax.tile([64 + LG, FD], bf16, tag="exp", name="exp_t")
        zsb = smax.tile([HPG, FD], f32, tag="zsb", name="zsb")
        bcs = [None, None]
        for q in range(2):
            emit_prods_half(tiles[0], tiles[1], pts, q)
        for ch in range(NCH):
            if ch % 2 == 0:
                bcs[ch // 2] = bcp.tile([128, K2 * HFD], bf16, tag="bc",
                                        name="bc_t")
            emit_softmax_chunk(ptiles, exp_t, zsb, bcs[ch // 2], ch)
            if ch % 2 == 1:
                emit_bc_half(exp_t, bcs[ch // 2], ch // 2)

        for g in range(NGRP):
            v3_cur = tiles[2]
            zsb_cur = zsb
            bcs_cur = bcs
            nxt = g + 1 < NGRP
            if nxt:
                tiles = load_group(g + 1)
                pts, ptiles = alloc_prods()
                exp_t = smax.tile([64 + LG, FD], bf16, tag="exp", name="exp_t")
                zsb = smax.tile([HPG, FD], f32, tag="zsb", name="zsb")
                bcs = [None, None]
            acc_t = smax.tile([128, FD], bf16, tag="acc", name="acc_t")
            for ch in range(NCH):
                if nxt:
                    if ch % 2 == 0:
                        emit_prods_half(tiles[0], tiles[1], pts, ch // 2)
                        bcs[ch // 2] = bcp.tile([128, K2 * HFD], bf16,
                                                tag="bc", name="bc_t")
                    emit_softmax_chunk(ptiles, exp_t, zsb, bcs[ch // 2], ch)
                pavs = emit_pav(bcs_cur[ch // 2], v3_cur, ch)
                emit_ksum(acc_t, ch, pavs)
                if nxt and ch % 2 == 1:
                    emit_bc_half(exp_t, bcs[ch // 2], ch // 2)
            nc.sync.dma_start(z_p[g * HPG:(g + 1) * HPG, :], zsb_cur[:])
            nc.sync.dma_start(out_p[g * 128:(g + 1) * 128, :], acc_t[:])

    nc.compile()
    return nc


def _get_nc():
    if "nc" not in _CACHE:
        _CACHE["nc"] = _build_nc()
    return _CACHE["nc"]


def build_in_maps(q, k, v):
    qb = np.asarray(q, np.float32).astype(BF16)
    kp = np.pad(np.asarray(k, np.float32),
                ((0, 0), (0, 0), (PAD, PAD), (PAD, PAD))).astype(BF16)
    vp = np.pad(np.asarray(v, np.float32),
                ((0, 0), (0, 0), (PAD, PAD), (PAD, PAD))).astype(BF16)
    selqk, selz, selbc = _CACHE.setdefault("sel", _build_sel_constants())
    in_maps = []
    for c in range(NCORES):
        b, half = divmod(c, 2)
        r0 = half * ROWS
        in_maps.append({
            "q": np.ascontiguousarray(qb[b, :, r0:r0 + ROWS, :]).reshape(D, FD),
            "k": np.ascontiguousarray(kp[b, :, r0:r0 + HROWS, :]).reshape(D, HROWS * WP),
            "v": np.ascontiguousarray(vp[b, :, r0:r0 + HROWS, :]).reshape(D, HROWS * WP),
            "selqk": selqk, "selz": selz, "selbc": selbc,
        })
    return in_maps


def kernel(q, k, v):
    in_maps = build_in_maps(q, k, v)
    nc = _get_nc()
    res = run_bass_kernel_spmd(nc, in_maps, core_ids=list(range(NCORES)))
    out = np.empty((B, H, W, D), np.float32)
    for c in range(NCORES):
        b, half = divmod(c, 2)
        r0 = half * ROWS
        # out dram is [D, FD] channel-major UNNORMALIZED bf16; z is [12, FD]
        ocm = res.results[c]["out"].astype(np.float32)
        z = res.results[c]["z"]                       # [12, FD] f32
        ocm /= np.repeat(z, HD, axis=0)               # [384, FD]
        out[b, r0:r0 + ROWS] = ocm.T.reshape(ROWS, W, D)
    return out
